# revision 25
# baseline (speedup 1.0000x reference)
"""Conv2d 3x3 VALID stride-1 kernel for Trainium2 (Bass/Tile), 8-core SPMD.

x: [32, 128, 112, 112] f32, weight: [256, 128, 3, 3] f32
out: [32, 256, 110, 110] f32

Strategy: 1-D Winograd F(8,3) along W + implicit GEMM over (Cin, kh).
The host precomputes the Winograd input transform t_p = B^T x along W
(10 planes of 14 j-positions per row, fp16) and the weight transform
g'_p = G w (fp16). Per output row-group the PE runs 10 planes x 3 kh
taps of width R*14 instead of the direct conv's 9 taps of width R*110
-- 2.36x fewer PE cycles (direct fp16 roofline 363.6 us -> 154 us
here). Nodes {0,±1,±2,±1/2,±3/4} keep the fp16 transform error at
~5.5e-3 rel (measured offline), well under the 2e-2 gate.

The m-planes accumulate in PSUM (fp32) and are evacuated to SBUF as
fp16 by the Scalar engine (p0..p4) and DVE (p5..p9) into per-row-group
slabs laid out [cout, ct, p, row, j]; each (row-group, ct, plane-half)
flushes to HBM as its own ~0.7MB descriptor as soon as its planes are
evacuated. Store throughput is ~(run bytes) x (~120 runs/us per
outstanding descriptor), so keeping ~4 descriptors in flight saturates
the ~300 GB/s HBM ceiling and the final drain after the last matmul
stays small. The A^T output combine runs on the host (fixed linear
postprocess).

Data-parallel over batch: 4 images per core, weights replicated.
"""

import numpy as np
from fractions import Fraction as Fr

import concourse.mybir as mybir
import concourse.tile as tile
from concourse import bacc
from concourse.bass_utils import run_bass_kernel_spmd

B, CIN, H, W = 32, 128, 112, 112
COUT, KH, KW = 256, 3, 3
OH, OW = H - KH + 1, W - KW + 1  # 110, 110
NCORES = 8
BPC = B // NCORES  # batches per core

M_TILE = 8   # Winograd F(8,3): 8 outputs per j-group
NP = 10      # m-planes (= M_TILE + KW - 1)
NJ = 14      # j-positions along W (8 outputs each, 8*14=112 >= 110)
WPAD = M_TILE * (NJ - 1) + NP  # 114: padded input width
F32 = mybir.dt.float32
FP16 = mybir.dt.float16

# Row-groups of the 110 output rows; R*NJ <= 512 (one PSUM bank).
ROW_CHUNKS = [28, 28, 28, 26]
# (row-group slabs; no half-image split)

# F(8,3) interpolation nodes (9 finite + infinity).
NODES = [0, 1, -1, 2, -2, Fr(1, 2), Fr(-1, 2), Fr(3, 4), Fr(-3, 4)]


def _cook_toom(m, r, points):
    """A^T [m x a], G [a x r], B^T [a x a] for F(m,r), a-1 finite points
    + infinity; y = A^T [(G w) * (B^T x)] (correlation). Exact rationals."""
    a = m + r - 1
    pts = [Fr(p) for p in points]
    denom = []
    for j in range(a - 1):
        dd = Fr(1)
        for l in range(a - 1):
            if l != j:
                dd *= pts[j] - pts[l]
        denom.append(dd)
    G = [[(pts[j] ** k) / denom[j] for k in range(r)] for j in range(a - 1)]
    G.append([Fr(0)] * (r - 1) + [Fr(1)])
    AT = [[pts[j] ** i for j in range(a - 1)] + [Fr(1) if i == m - 1 else Fr(0)]
          for i in range(m)]

    def polymul(c1, c2):
        out = [Fr(0)] * (len(c1) + len(c2) - 1)
        for i, v1 in enumerate(c1):
            for j, v2 in enumerate(c2):
                out[i + j] += v1 * v2
        return out

    BT = []
    for j in range(a - 1):
        c = [Fr(1)]
        for l in range(a - 1):
            if l != j:
                c = polymul(c, [-pts[l], Fr(1)])
        BT.append(c + [Fr(0)] * (a - len(c)))
    c = [Fr(1)]
    for l in range(a - 1):
        c = polymul(c, [-pts[l], Fr(1)])
    BT.append(c + [Fr(0)] * (a - len(c)))

    tofl = lambda M: np.array([[float(v) for v in row] for row in M])
    return tofl(AT), tofl(G), tofl(BT)


AT_MAT, G_MAT, BT_MAT = _cook_toom(M_TILE, KW, NODES)

_CACHE = {}


def _build_nc():
    nc = bacc.Bacc("TRN2", target_bir_lowering=False, debug=False)

    # Plane-major input layout: the matmul rhs slice [rows, NJ] is ONE
    # contiguous 392-element run per partition (short 14-element runs
    # measured +60ns/MM of AP-walk overhead on the PE).
    t_d = nc.dram_tensor("t", [BPC, CIN, NP, H, NJ], FP16, kind="ExternalInput")
    # ct-major weight layout: each cout-half loads contiguously.
    w_d = nc.dram_tensor("w", [2, CIN, NP, KH, 128], FP16, kind="ExternalInput")
    # Blocked m output: per (row-group) a contiguous [p, rows, j] block
    # per cout, so each store descriptor is 128 runs of ~7.8KB (DMA
    # dispatch is serial at ~4.3ns/run -- throughput = run_bytes/4.3ns;
    # short runs cap stores far below the ~300 GB/s HBM ceiling).
    # Host un-blocks before the A^T combine.
    o_d = nc.dram_tensor(
        "o", [BPC, COUT, NP * OH * NJ], FP16, kind="ExternalOutput"
    )

    from concourse.bass import _add_dep_helper

    # Prefetch chunking of images b >= 1: plane-group slices (full rows,
    # 3136B contiguous runs) paced against the previous image's
    # row-groups; front-loaded so all planes land before the image
    # starts. The gi=2 slot issues two descriptors (planes 6:8, 8:10)
    # for concurrent expansion.
    PF_PLANES = {0: [(0, 3)], 1: [(3, 6)], 2: [(6, 8), (8, 10)], 3: []}

    with tile.TileContext(nc) as tc:
        with (
            tc.tile_pool(name="wpool", bufs=1) as wpool,
            tc.tile_pool(name="xpool", bufs=2) as xpool,
            tc.tile_pool(name="mpool", bufs=5) as mpool,
            tc.tile_pool(name="psum", bufs=8, space="PSUM") as psum,
        ):
            # PE pre-warm: a dependency-free matmul burst keeps the HAM
            # activity monitor busy from boot until the first real
            # matmul's inputs land (~11-12us incl. the ~7us framework
            # preamble); any idle window >3.4us re-throttles the PE clock
            # to 1.2GHz for the next ~14us.
            scratch = wpool.tile([128, 256], FP16, name="warm_scratch")
            nc.vector.memset(scratch[:], 0)
            ps_warm = psum.tile([128, 256], F32, name="warm_psum", tag="ps")
            for _ in range(24):
                nc.tensor.matmul(
                    ps_warm[:], scratch[:, 0:128], scratch[:],
                    start=True, stop=True, skip_group_check=True,
                )

            wr = wpool.tile([CIN, 2, NP, KH, 128], FP16)

            # Startup ordering by first-matmul need (DMA is the startup
            # bottleneck at ~300 GB/s aggregate): plane-0 ct0 weights,
            # low planes of group-0 rows, remaining ct0 weights, high
            # planes, ct1 weights, then the rest of image 0 row-chunked.
            xtiles = [xpool.tile([CIN, NP, H, NJ], FP16, tag="x", name="x0")]
            nc.gpsimd.dma_start(wr[:, 0, 0], w_d[0, :, 0])
            nc.gpsimd.dma_start(
                xtiles[0][:, 0:5, 0:30, :], t_d[0, :, 0:5, 0:30, :]
            )
            nc.gpsimd.dma_start(wr[:, 0, 1:NP], w_d[0, :, 1:NP])
            nc.gpsimd.dma_start(
                xtiles[0][:, 5:NP, 0:30, :], t_d[0, :, 5:NP, 0:30, :]
            )
            nc.gpsimd.dma_start(wr[:, 1], w_d[1])
            nc.gpsimd.dma_start(
                xtiles[0][:, :, 30:58, :], t_d[0, :, :, 30:58, :]
            )
            nc.gpsimd.dma_start(
                xtiles[0][:, :, 58:86, :], t_d[0, :, :, 58:86, :]
            )
            nc.gpsimd.dma_start(
                xtiles[0][:, :, 86:112, :], t_d[0, :, :, 86:112, :]
            )

            for b in range(BPC):
                xr = xtiles[b]
                last_img = b == BPC - 1
                if not last_img:
                    xtiles.append(
                        xpool.tile(
                            [CIN, NP, H, NJ], FP16, tag="x", name=f"x{b+1}"
                        )
                    )
                oh = 0
                for gi, R in enumerate(ROW_CHUNKS):
                    # Per-row-group slab [cout, ct, p, row, j], 5-deep
                    # pool: WAR against its own store is ~4 row-groups
                    # (~40us) away. Each (ct, plane-half) flushes to its
                    # contiguous block of o_d as soon as its planes are
                    # evacuated.
                    off = NP * NJ * oh
                    slab = mpool.tile([128, 2, NP, R, NJ], FP16, tag="m",
                                      name=f"m{b}_{gi}")
                    last_cast = None
                    for ct in range(2):
                        ps = []
                        for p in range(NP):
                            pst = psum.tile([128, R, NJ], F32, tag="ps")
                            ps.append(pst)
                            for kh in range(KH):
                                nc.tensor.matmul(
                                    pst[:],
                                    wr[:, ct, p, kh, :],
                                    xr[:, p, oh + kh : oh + kh + R, :],
                                    start=(kh == 0),
                                    stop=(kh == KH - 1),
                                )
                        # Evacuate: ACT p0..p4, DVE p5..p9 (fp32 -> fp16).
                        for p in range(5):
                            nc.scalar.copy(slab[:, ct, p], ps[p][:])
                        co0 = ct * 128
                        half = 5 * R * NJ
                        nc.sync.dma_start(
                            o_d[b, co0 : co0 + 128, off : off + half],
                            slab[:, ct, 0:5],
                        )
                        for p in range(5, NP):
                            last_cast = nc.vector.tensor_copy(
                                slab[:, ct, p], ps[p][:]
                            )
                        nc.sync.dma_start(
                            o_d[b, co0 : co0 + 128, off + half : off + 2 * half],
                            slab[:, ct, 5:NP],
                        )
                    oh += R
                    if not last_img:
                        for p0, p1 in PF_PLANES[gi]:
                            dma = nc.gpsimd.dma_start(
                                xtiles[b + 1][:, p0:p1, :, :],
                                t_d[b + 1, :, p0:p1, :, :],
                            )
                            _add_dep_helper(
                                dma.ins,
                                last_cast.ins,
                                sync=True,
                                reason="pace input prefetch vs compute",
                            )

    nc.compile()
    return nc


def _get_nc():
    if "nc" not in _CACHE:
        _CACHE["nc"] = _build_nc()
    return _CACHE["nc"]


LAST_RESULT = None


def _host_transform_x(x):
    """x[32,128,112,112] f32 -> t[32,128,10,112,14] fp16 (B^T x along W,
    plane-major)."""
    x = np.asarray(x, dtype=np.float32)
    bt = np.ascontiguousarray(BT_MAT.T, dtype=np.float32)  # [10 taps, 10 p]
    t = np.empty((B, CIN, NP, H, NJ), dtype=np.float16)
    for b0 in range(0, B, 8):
        xp = np.zeros((8, CIN, H, WPAD), dtype=np.float32)
        xp[:, :, :, :W] = x[b0 : b0 + 8]
        # D[b,c,h,j,k] = xp[b,c,h,8j+k]
        D = np.lib.stride_tricks.sliding_window_view(xp, NP, axis=3)[
            :, :, :, :: M_TILE
        ]  # [8, C, H, 14, 10]
        tc = D @ bt  # [8, C, H, 14, 10planes]
        t[b0 : b0 + 8] = tc.transpose(0, 1, 4, 2, 3)
    return t


def _host_combine(mblk):
    """Blocked m [B, COUT, NP*OH*NJ] fp16 -> out [B, COUT, OH, 110] f32.

    Un-blocks per row-group ([p, rows, j] contiguous per cout), then
    applies A^T along the plane axis."""
    at = np.ascontiguousarray(AT_MAT, dtype=np.float32)  # [8, 10]
    nb = mblk.shape[0]
    m = np.empty((nb, COUT, NP, OH, NJ), dtype=np.float16)
    oh = 0
    for R in ROW_CHUNKS:
        off = NP * NJ * oh
        blk = mblk[:, :, off : off + NP * R * NJ].reshape(nb, COUT, NP, R, NJ)
        m[:, :, :, oh : oh + R, :] = blk
        oh += R
    out = np.empty((nb, COUT, OH, OW), dtype=np.float32)
    for b in range(nb):
        mb = m[b].astype(np.float32)  # [COUT, 10, OH, 14]
        o = np.tensordot(mb, at, axes=([1], [1]))  # [COUT, OH, 14, 8]
        out[b] = o.reshape(COUT, OH, M_TILE * NJ)[:, :, :OW]
    return out


def kernel(x, weight, trace=False):
    global LAST_RESULT
    t = _host_transform_x(x)
    # weight [Cout,Cin,3,3] -> g'[cin, p, kh, cout] = sum_kw G[p,kw] w
    w64 = np.asarray(weight, dtype=np.float64)
    wt = np.einsum("pw,ochw->cpho", G_MAT, w64).astype(np.float16)
    # [cin, p, kh, cout] -> [ct, cin, p, kh, 128]
    wt = np.ascontiguousarray(
        wt.reshape(CIN, NP, KH, 2, 128).transpose(3, 0, 1, 2, 4)
    )

    nc = _get_nc()
    in_maps = [
        {"t": t[i * BPC : (i + 1) * BPC], "w": wt} for i in range(NCORES)
    ]
    res = run_bass_kernel_spmd(
        nc, in_maps, core_ids=list(range(NCORES)), trace=trace
    )
    LAST_RESULT = res
    m = np.concatenate([r["o"] for r in res.results], axis=0)
    return _host_combine(m)


# revision 28
# speedup vs baseline: 1.0254x; 1.0254x over previous
"""Conv2d 3x3 VALID stride-1 kernel for Trainium2 (Bass/Tile), 8-core SPMD.

x: [32, 128, 112, 112] f32, weight: [256, 128, 3, 3] f32
out: [32, 256, 110, 110] f32

Strategy: 1-D Winograd F(8,3) along W + implicit GEMM over (Cin, kh).
The host precomputes the Winograd input transform t_p = B^T x along W
(10 planes of 14 j-positions per row, fp16) and the weight transform
g'_p = G w (fp16). Per output row-group the PE runs 10 planes x 3 kh
taps of width R*14 instead of the direct conv's 9 taps of width R*110
-- 2.36x fewer PE cycles (direct fp16 roofline 363.6 us -> 154 us
here). Nodes {0,±1,±2,±1/2,±3/4} keep the fp16 transform error at
~5.5e-3 rel (measured offline), well under the 2e-2 gate.

The m-planes accumulate in PSUM (fp32) and are evacuated to SBUF as
fp16 by the Scalar engine (p0..p4) and DVE (p5..p9) into per-row-group
slabs laid out [cout, ct, p, row, j]; each (row-group, ct, plane-half)
flushes to HBM as its own ~0.7MB descriptor as soon as its planes are
evacuated. Store throughput is ~(run bytes) x (~120 runs/us per
outstanding descriptor), so keeping ~4 descriptors in flight saturates
the ~300 GB/s HBM ceiling and the final drain after the last matmul
stays small. The A^T output combine runs on the host (fixed linear
postprocess).

Data-parallel over batch: 4 images per core, weights replicated.
"""

import numpy as np
from fractions import Fraction as Fr

import concourse.mybir as mybir
import concourse.tile as tile
from concourse import bacc
from concourse.bass_utils import run_bass_kernel_spmd

B, CIN, H, W = 32, 128, 112, 112
COUT, KH, KW = 256, 3, 3
OH, OW = H - KH + 1, W - KW + 1  # 110, 110
NCORES = 8
BPC = B // NCORES  # batches per core

M_TILE = 8   # Winograd F(8,3): 8 outputs per j-group
NP = 10      # m-planes (= M_TILE + KW - 1)
NJ = 14      # j-positions along W (8 outputs each, 8*14=112 >= 110)
WPAD = M_TILE * (NJ - 1) + NP  # 114: padded input width
F32 = mybir.dt.float32
FP16 = mybir.dt.float16

# Row-groups of the 110 output rows; R*NJ <= 512 (one PSUM bank).
ROW_CHUNKS = [28, 28, 28, 26]
# (row-group slabs; no half-image split)

# F(8,3) interpolation nodes (9 finite + infinity).
NODES = [0, 1, -1, 2, -2, Fr(1, 2), Fr(-1, 2), Fr(3, 4), Fr(-3, 4)]


def _cook_toom(m, r, points):
    """A^T [m x a], G [a x r], B^T [a x a] for F(m,r), a-1 finite points
    + infinity; y = A^T [(G w) * (B^T x)] (correlation). Exact rationals."""
    a = m + r - 1
    pts = [Fr(p) for p in points]
    denom = []
    for j in range(a - 1):
        dd = Fr(1)
        for l in range(a - 1):
            if l != j:
                dd *= pts[j] - pts[l]
        denom.append(dd)
    G = [[(pts[j] ** k) / denom[j] for k in range(r)] for j in range(a - 1)]
    G.append([Fr(0)] * (r - 1) + [Fr(1)])
    AT = [[pts[j] ** i for j in range(a - 1)] + [Fr(1) if i == m - 1 else Fr(0)]
          for i in range(m)]

    def polymul(c1, c2):
        out = [Fr(0)] * (len(c1) + len(c2) - 1)
        for i, v1 in enumerate(c1):
            for j, v2 in enumerate(c2):
                out[i + j] += v1 * v2
        return out

    BT = []
    for j in range(a - 1):
        c = [Fr(1)]
        for l in range(a - 1):
            if l != j:
                c = polymul(c, [-pts[l], Fr(1)])
        BT.append(c + [Fr(0)] * (a - len(c)))
    c = [Fr(1)]
    for l in range(a - 1):
        c = polymul(c, [-pts[l], Fr(1)])
    BT.append(c + [Fr(0)] * (a - len(c)))

    tofl = lambda M: np.array([[float(v) for v in row] for row in M])
    return tofl(AT), tofl(G), tofl(BT)


AT_MAT, G_MAT, BT_MAT = _cook_toom(M_TILE, KW, NODES)

_CACHE = {}


def _build_nc():
    nc = bacc.Bacc("TRN2", target_bir_lowering=False, debug=False)

    # Plane-major input layout: the matmul rhs slice [rows, NJ] is ONE
    # contiguous 392-element run per partition (short 14-element runs
    # measured +60ns/MM of AP-walk overhead on the PE).
    t_d = nc.dram_tensor("t", [BPC, CIN, NP, H, NJ], FP16, kind="ExternalInput")
    # ct-major weight layout: each cout-half loads contiguously.
    w_d = nc.dram_tensor("w", [2, CIN, NP, KH, 128], FP16, kind="ExternalInput")
    # Blocked m output: per (row-group) a contiguous [p, rows, j] block
    # per cout, so each store descriptor is 128 runs of ~7.8KB (DMA
    # dispatch is serial at ~4.3ns/run -- throughput = run_bytes/4.3ns;
    # short runs cap stores far below the ~300 GB/s HBM ceiling).
    # Host un-blocks before the A^T combine.
    o_d = nc.dram_tensor(
        "o", [BPC, COUT, NP * OH * NJ], FP16, kind="ExternalOutput"
    )

    from concourse.bass import _add_dep_helper

    # Prefetch chunking of images b >= 1: plane-group slices (full rows,
    # 3136B contiguous runs) paced against the previous image's
    # row-groups; front-loaded so all planes land before the image
    # starts. The gi=2 slot issues two descriptors (planes 6:8, 8:10)
    # for concurrent expansion.
    PF_PLANES = {0: [(0, 3)], 1: [(3, 6)], 2: [(6, 8), (8, 10)], 3: []}

    with tile.TileContext(nc) as tc:
        with (
            tc.tile_pool(name="wpool", bufs=1) as wpool,
            tc.tile_pool(name="xpool", bufs=2) as xpool,
            tc.tile_pool(name="mpool", bufs=5) as mpool,
            tc.tile_pool(name="psum", bufs=8, space="PSUM") as psum,
        ):
            # PE pre-warm: a dependency-free matmul burst keeps the HAM
            # activity monitor busy from boot until the first real
            # matmul's inputs land (~11-12us incl. the ~7us framework
            # preamble); any idle window >3.4us re-throttles the PE clock
            # to 1.2GHz for the next ~14us.
            scratch = wpool.tile([128, 256], FP16, name="warm_scratch")
            nc.vector.memset(scratch[:], 0)
            ps_warm = psum.tile([128, 256], F32, name="warm_psum", tag="ps")
            for _ in range(12):
                nc.tensor.matmul(
                    ps_warm[:], scratch[:, 0:128], scratch[:],
                    start=True, stop=True, skip_group_check=True,
                )

            wr = wpool.tile([CIN, 2, NP, KH, 128], FP16)

            # Startup ordering by first-matmul need (DMA is the startup
            # bottleneck at ~300 GB/s aggregate). Image 0 loads per
            # plane-group over FULL rows: (p, h, j) is contiguous per
            # partition, so each descriptor is a single ~KB-scale run --
            # dispatch-cheap -- and image 0's plane-outer compute loop
            # consumes planes in exactly this order.
            xtiles = [xpool.tile([CIN, NP, H, NJ], FP16, tag="x", name="x0")]
            nc.gpsimd.dma_start(wr[:, 0, 0], w_d[0, :, 0])
            nc.gpsimd.dma_start(xtiles[0][:, 0:1], t_d[0, :, 0:1])
            nc.gpsimd.dma_start(wr[:, 0, 1:NP], w_d[0, :, 1:NP])
            nc.gpsimd.dma_start(xtiles[0][:, 1:3], t_d[0, :, 1:3])
            nc.gpsimd.dma_start(xtiles[0][:, 3:5], t_d[0, :, 3:5])
            nc.gpsimd.dma_start(xtiles[0][:, 5:7], t_d[0, :, 5:7])
            nc.gpsimd.dma_start(xtiles[0][:, 7:NP], t_d[0, :, 7:NP])
            nc.gpsimd.dma_start(wr[:, 1], w_d[1])

            for b in range(BPC):
                xr = xtiles[b]
                last_img = b == BPC - 1
                if not last_img:
                    xtiles.append(
                        xpool.tile(
                            [CIN, NP, H, NJ], FP16, tag="x", name=f"x{b+1}"
                        )
                    )
                if b == 0:
                    # Image 0 runs plane-outer: each plane's full column
                    # of row-groups computes as soon as that plane's data
                    # lands (the startup loads stream plane-major), so the
                    # PE starts ~4us earlier and stays busy through the
                    # DMA-throttled first image. Image-1 prefetch paces
                    # off plane milestones.
                    slabs0 = [
                        mpool.tile([128, 2, NP, R, NJ], FP16, tag="m",
                                   name=f"m0_{gi}")
                        for gi, R in enumerate(ROW_CHUNKS)
                    ]
                    PACE0 = {(0, 4): (0, 3), (0, 7): (3, 6),
                             (1, 2): (6, 8), (1, 5): (8, 10)}
                    for ct in range(2):
                        co0 = ct * 128
                        for p in range(NP):
                            oh = 0
                            ev = None
                            for gi, R in enumerate(ROW_CHUNKS):
                                pst = psum.tile([128, R, NJ], F32, tag="ps")
                                for kh in range(KH):
                                    nc.tensor.matmul(
                                        pst[:],
                                        wr[:, ct, p, kh, :],
                                        xr[:, p, oh + kh : oh + kh + R, :],
                                        start=(kh == 0),
                                        stop=(kh == KH - 1),
                                    )
                                if p < 5:
                                    ev = nc.scalar.copy(
                                        slabs0[gi][:, ct, p], pst[:]
                                    )
                                else:
                                    ev = nc.vector.tensor_copy(
                                        slabs0[gi][:, ct, p], pst[:]
                                    )
                                oh += R
                            if p == 4 or p == NP - 1:
                                ph0, ph1 = (0, 5) if p == 4 else (5, NP)
                                oh = 0
                                for gi, R in enumerate(ROW_CHUNKS):
                                    off = NP * NJ * oh + ph0 * R * NJ
                                    sz = 5 * R * NJ
                                    nc.sync.dma_start(
                                        o_d[0, co0 : co0 + 128,
                                            off : off + sz],
                                        slabs0[gi][:, ct, ph0:ph1],
                                    )
                                    oh += R
                            if (ct, p) in PACE0:
                                p0, p1 = PACE0[(ct, p)]
                                dma = nc.gpsimd.dma_start(
                                    xtiles[1][:, p0:p1, :, :],
                                    t_d[1, :, p0:p1, :, :],
                                )
                                _add_dep_helper(
                                    dma.ins, ev.ins, sync=True,
                                    reason="pace image-1 prefetch",
                                )
                    continue
                oh = 0
                for gi, R in enumerate(ROW_CHUNKS):
                    # Per-row-group slab [cout, ct, p, row, j], 5-deep
                    # pool: WAR against its own store is ~4 row-groups
                    # (~40us) away. Each (ct, plane-half) flushes to its
                    # contiguous block of o_d as soon as its planes are
                    # evacuated.
                    off = NP * NJ * oh
                    slab = mpool.tile([128, 2, NP, R, NJ], FP16, tag="m",
                                      name=f"m{b}_{gi}")
                    last_cast = None
                    for ct in range(2):
                        ps = []
                        for p in range(NP):
                            pst = psum.tile([128, R, NJ], F32, tag="ps")
                            ps.append(pst)
                            for kh in range(KH):
                                nc.tensor.matmul(
                                    pst[:],
                                    wr[:, ct, p, kh, :],
                                    xr[:, p, oh + kh : oh + kh + R, :],
                                    start=(kh == 0),
                                    stop=(kh == KH - 1),
                                )
                        # Evacuate: ACT p0..p4, DVE p5..p9 (fp32 -> fp16).
                        for p in range(5):
                            nc.scalar.copy(slab[:, ct, p], ps[p][:])
                        co0 = ct * 128
                        half = 5 * R * NJ
                        nc.sync.dma_start(
                            o_d[b, co0 : co0 + 128, off : off + half],
                            slab[:, ct, 0:5],
                        )
                        for p in range(5, NP):
                            last_cast = nc.vector.tensor_copy(
                                slab[:, ct, p], ps[p][:]
                            )
                        nc.sync.dma_start(
                            o_d[b, co0 : co0 + 128, off + half : off + 2 * half],
                            slab[:, ct, 5:NP],
                        )
                    oh += R
                    if not last_img:
                        for p0, p1 in PF_PLANES[gi]:
                            dma = nc.gpsimd.dma_start(
                                xtiles[b + 1][:, p0:p1, :, :],
                                t_d[b + 1, :, p0:p1, :, :],
                            )
                            _add_dep_helper(
                                dma.ins,
                                last_cast.ins,
                                sync=True,
                                reason="pace input prefetch vs compute",
                            )

    nc.compile()
    return nc


def _get_nc():
    if "nc" not in _CACHE:
        _CACHE["nc"] = _build_nc()
    return _CACHE["nc"]


LAST_RESULT = None


def _host_transform_x(x):
    """x[32,128,112,112] f32 -> t[32,128,10,112,14] fp16 (B^T x along W,
    plane-major)."""
    x = np.asarray(x, dtype=np.float32)
    bt = np.ascontiguousarray(BT_MAT.T, dtype=np.float32)  # [10 taps, 10 p]
    t = np.empty((B, CIN, NP, H, NJ), dtype=np.float16)
    for b0 in range(0, B, 8):
        xp = np.zeros((8, CIN, H, WPAD), dtype=np.float32)
        xp[:, :, :, :W] = x[b0 : b0 + 8]
        # D[b,c,h,j,k] = xp[b,c,h,8j+k]
        D = np.lib.stride_tricks.sliding_window_view(xp, NP, axis=3)[
            :, :, :, :: M_TILE
        ]  # [8, C, H, 14, 10]
        tc = D @ bt  # [8, C, H, 14, 10planes]
        t[b0 : b0 + 8] = tc.transpose(0, 1, 4, 2, 3)
    return t


def _host_combine(mblk):
    """Blocked m [B, COUT, NP*OH*NJ] fp16 -> out [B, COUT, OH, 110] f32.

    Un-blocks per row-group ([p, rows, j] contiguous per cout), then
    applies A^T along the plane axis."""
    at = np.ascontiguousarray(AT_MAT, dtype=np.float32)  # [8, 10]
    nb = mblk.shape[0]
    m = np.empty((nb, COUT, NP, OH, NJ), dtype=np.float16)
    oh = 0
    for R in ROW_CHUNKS:
        off = NP * NJ * oh
        blk = mblk[:, :, off : off + NP * R * NJ].reshape(nb, COUT, NP, R, NJ)
        m[:, :, :, oh : oh + R, :] = blk
        oh += R
    out = np.empty((nb, COUT, OH, OW), dtype=np.float32)
    for b in range(nb):
        mb = m[b].astype(np.float32)  # [COUT, 10, OH, 14]
        o = np.tensordot(mb, at, axes=([1], [1]))  # [COUT, OH, 14, 8]
        out[b] = o.reshape(COUT, OH, M_TILE * NJ)[:, :, :OW]
    return out


def kernel(x, weight, trace=False):
    global LAST_RESULT
    t = _host_transform_x(x)
    # weight [Cout,Cin,3,3] -> g'[cin, p, kh, cout] = sum_kw G[p,kw] w
    w64 = np.asarray(weight, dtype=np.float64)
    wt = np.einsum("pw,ochw->cpho", G_MAT, w64).astype(np.float16)
    # [cin, p, kh, cout] -> [ct, cin, p, kh, 128]
    wt = np.ascontiguousarray(
        wt.reshape(CIN, NP, KH, 2, 128).transpose(3, 0, 1, 2, 4)
    )

    nc = _get_nc()
    in_maps = [
        {"t": t[i * BPC : (i + 1) * BPC], "w": wt} for i in range(NCORES)
    ]
    res = run_bass_kernel_spmd(
        nc, in_maps, core_ids=list(range(NCORES)), trace=trace
    )
    LAST_RESULT = res
    m = np.concatenate([r["o"] for r in res.results], axis=0)
    return _host_combine(m)


# revision 29
# speedup vs baseline: 1.0397x; 1.0139x over previous
"""Conv2d 3x3 VALID stride-1 kernel for Trainium2 (Bass/Tile), 8-core SPMD.

x: [32, 128, 112, 112] f32, weight: [256, 128, 3, 3] f32
out: [32, 256, 110, 110] f32

Strategy: 1-D Winograd F(8,3) along W + implicit GEMM over (Cin, kh).
The host precomputes the Winograd input transform t_p = B^T x along W
(10 planes of 14 j-positions per row, fp16) and the weight transform
g'_p = G w (fp16). Per output row-group the PE runs 10 planes x 3 kh
taps of width R*14 instead of the direct conv's 9 taps of width R*110
-- 2.36x fewer PE cycles (direct fp16 roofline 363.6 us -> 154 us
here). Nodes {0,±1,±2,±1/2,±3/4} keep the fp16 transform error at
~5.5e-3 rel (measured offline), well under the 2e-2 gate.

The m-planes accumulate in PSUM (fp32) and are evacuated to SBUF as
fp16 by the Scalar engine (p0..p4) and DVE (p5..p9) into per-row-group
slabs laid out [cout, ct, p, row, j]; each (row-group, ct, plane-half)
flushes to HBM as its own ~0.7MB descriptor as soon as its planes are
evacuated. Store throughput is ~(run bytes) x (~120 runs/us per
outstanding descriptor), so keeping ~4 descriptors in flight saturates
the ~300 GB/s HBM ceiling and the final drain after the last matmul
stays small. The A^T output combine runs on the host (fixed linear
postprocess).

Data-parallel over batch: 4 images per core, weights replicated.
"""

import numpy as np
from fractions import Fraction as Fr

import concourse.mybir as mybir
import concourse.tile as tile
from concourse import bacc
from concourse.bass_utils import run_bass_kernel_spmd

B, CIN, H, W = 32, 128, 112, 112
COUT, KH, KW = 256, 3, 3
OH, OW = H - KH + 1, W - KW + 1  # 110, 110
NCORES = 8
BPC = B // NCORES  # batches per core

M_TILE = 8   # Winograd F(8,3): 8 outputs per j-group
NP = 10      # m-planes (= M_TILE + KW - 1)
NJ = 14      # j-positions along W (8 outputs each, 8*14=112 >= 110)
WPAD = M_TILE * (NJ - 1) + NP  # 114: padded input width
F32 = mybir.dt.float32
FP16 = mybir.dt.float16

# Row-groups of the 110 output rows; R*NJ <= 512 (one PSUM bank).
ROW_CHUNKS = [28, 28, 28, 26]
# (row-group slabs; no half-image split)

# F(8,3) interpolation nodes (9 finite + infinity).
NODES = [0, 1, -1, 2, -2, Fr(1, 2), Fr(-1, 2), Fr(3, 4), Fr(-3, 4)]


def _cook_toom(m, r, points):
    """A^T [m x a], G [a x r], B^T [a x a] for F(m,r), a-1 finite points
    + infinity; y = A^T [(G w) * (B^T x)] (correlation). Exact rationals."""
    a = m + r - 1
    pts = [Fr(p) for p in points]
    denom = []
    for j in range(a - 1):
        dd = Fr(1)
        for l in range(a - 1):
            if l != j:
                dd *= pts[j] - pts[l]
        denom.append(dd)
    G = [[(pts[j] ** k) / denom[j] for k in range(r)] for j in range(a - 1)]
    G.append([Fr(0)] * (r - 1) + [Fr(1)])
    AT = [[pts[j] ** i for j in range(a - 1)] + [Fr(1) if i == m - 1 else Fr(0)]
          for i in range(m)]

    def polymul(c1, c2):
        out = [Fr(0)] * (len(c1) + len(c2) - 1)
        for i, v1 in enumerate(c1):
            for j, v2 in enumerate(c2):
                out[i + j] += v1 * v2
        return out

    BT = []
    for j in range(a - 1):
        c = [Fr(1)]
        for l in range(a - 1):
            if l != j:
                c = polymul(c, [-pts[l], Fr(1)])
        BT.append(c + [Fr(0)] * (a - len(c)))
    c = [Fr(1)]
    for l in range(a - 1):
        c = polymul(c, [-pts[l], Fr(1)])
    BT.append(c + [Fr(0)] * (a - len(c)))

    tofl = lambda M: np.array([[float(v) for v in row] for row in M])
    return tofl(AT), tofl(G), tofl(BT)


AT_MAT, G_MAT, BT_MAT = _cook_toom(M_TILE, KW, NODES)

_CACHE = {}


def _build_nc():
    nc = bacc.Bacc("TRN2", target_bir_lowering=False, debug=False)

    # Plane-major input layout: the matmul rhs slice [rows, NJ] is ONE
    # contiguous 392-element run per partition (short 14-element runs
    # measured +60ns/MM of AP-walk overhead on the PE).
    t_d = nc.dram_tensor("t", [BPC, CIN, NP, H, NJ], FP16, kind="ExternalInput")
    # ct-major weight layout: each cout-half loads contiguously.
    w_d = nc.dram_tensor("w", [2, CIN, NP, KH, 128], FP16, kind="ExternalInput")
    # Blocked m output: per (row-group) a contiguous [p, rows, j] block
    # per cout, so each store descriptor is 128 runs of ~7.8KB (DMA
    # dispatch is serial at ~4.3ns/run -- throughput = run_bytes/4.3ns;
    # short runs cap stores far below the ~300 GB/s HBM ceiling).
    # Host un-blocks before the A^T combine.
    o_d = nc.dram_tensor(
        "o", [BPC, COUT, NP * OH * NJ], FP16, kind="ExternalOutput"
    )

    from concourse.bass import _add_dep_helper

    # Prefetch chunking of images b >= 1: plane-group slices (full rows,
    # 3136B contiguous runs) paced against the previous image's
    # row-groups; front-loaded so all planes land before the image
    # starts. The gi=2 slot issues two descriptors (planes 6:8, 8:10)
    # for concurrent expansion.
    PF_PLANES = {0: [(0, 3)], 1: [(3, 6)], 2: [(6, 8), (8, 10)], 3: []}

    with tile.TileContext(nc) as tc:
        with (
            tc.tile_pool(name="wpool", bufs=1) as wpool,
            tc.tile_pool(name="xpool", bufs=2) as xpool,
            tc.tile_pool(name="mpool", bufs=5) as mpool,
            tc.tile_pool(name="psum", bufs=8, space="PSUM") as psum,
        ):
            # PE pre-warm: a dependency-free matmul burst keeps the HAM
            # activity monitor busy from boot until the first real
            # matmul's inputs land (~11-12us incl. the ~7us framework
            # preamble); any idle window >3.4us re-throttles the PE clock
            # to 1.2GHz for the next ~14us.
            scratch = wpool.tile([128, 256], FP16, name="warm_scratch")
            nc.vector.memset(scratch[:], 0)
            ps_warm = psum.tile([128, 256], F32, name="warm_psum", tag="ps")
            for _ in range(28):
                nc.tensor.matmul(
                    ps_warm[:], scratch[:, 0:128], scratch[:],
                    start=True, stop=True, skip_group_check=True,
                )

            wr = wpool.tile([CIN, 2, NP, KH, 128], FP16)

            # Startup ordering by first-matmul need (DMA is the startup
            # bottleneck at ~300 GB/s aggregate). Image 0 loads per
            # plane-group over FULL rows: (p, h, j) is contiguous per
            # partition, so each descriptor is a single ~KB-scale run --
            # dispatch-cheap -- and image 0's plane-outer compute loop
            # consumes planes in exactly this order.
            xtiles = [xpool.tile([CIN, NP, H, NJ], FP16, tag="x", name="x0")]
            nc.gpsimd.dma_start(wr[:, 0, 0], w_d[0, :, 0])
            nc.gpsimd.dma_start(xtiles[0][:, 0:1], t_d[0, :, 0:1])
            nc.gpsimd.dma_start(wr[:, 0, 1:NP], w_d[0, :, 1:NP])
            nc.gpsimd.dma_start(xtiles[0][:, 1:3], t_d[0, :, 1:3])
            nc.gpsimd.dma_start(xtiles[0][:, 3:5], t_d[0, :, 3:5])
            nc.gpsimd.dma_start(xtiles[0][:, 5:7], t_d[0, :, 5:7])
            nc.gpsimd.dma_start(xtiles[0][:, 7:NP], t_d[0, :, 7:NP])
            nc.gpsimd.dma_start(wr[:, 1], w_d[1])

            for b in range(BPC):
                xr = xtiles[b]
                last_img = b == BPC - 1
                if not last_img:
                    xtiles.append(
                        xpool.tile(
                            [CIN, NP, H, NJ], FP16, tag="x", name=f"x{b+1}"
                        )
                    )
                if b == 0:
                    # Image 0 runs plane-outer: each plane's full column
                    # of row-groups computes as soon as that plane's data
                    # lands (the startup loads stream plane-major), so the
                    # PE starts ~4us earlier and stays busy through the
                    # DMA-throttled first image. Image-1 prefetch paces
                    # off plane milestones.
                    slabs0 = [
                        mpool.tile([128, 2, NP, R, NJ], FP16, tag="m",
                                   name=f"m0_{gi}")
                        for gi, R in enumerate(ROW_CHUNKS)
                    ]
                    PACE0 = {(0, 4): (0, 3), (0, 7): (3, 6),
                             (1, 2): (6, 8), (1, 5): (8, 10)}
                    for ct in range(2):
                        co0 = ct * 128
                        for p in range(NP):
                            oh = 0
                            ev = None
                            for gi, R in enumerate(ROW_CHUNKS):
                                pst = psum.tile([128, R, NJ], F32, tag="ps")
                                for kh in range(KH):
                                    nc.tensor.matmul(
                                        pst[:],
                                        wr[:, ct, p, kh, :],
                                        xr[:, p, oh + kh : oh + kh + R, :],
                                        start=(kh == 0),
                                        stop=(kh == KH - 1),
                                    )
                                if p < 5:
                                    ev = nc.scalar.copy(
                                        slabs0[gi][:, ct, p], pst[:]
                                    )
                                else:
                                    ev = nc.vector.tensor_copy(
                                        slabs0[gi][:, ct, p], pst[:]
                                    )
                                oh += R
                            if p == 4 or p == NP - 1:
                                ph0, ph1 = (0, 5) if p == 4 else (5, NP)
                                oh = 0
                                for gi, R in enumerate(ROW_CHUNKS):
                                    off = NP * NJ * oh + ph0 * R * NJ
                                    sz = 5 * R * NJ
                                    nc.sync.dma_start(
                                        o_d[0, co0 : co0 + 128,
                                            off : off + sz],
                                        slabs0[gi][:, ct, ph0:ph1],
                                    )
                                    oh += R
                            if (ct, p) in PACE0:
                                p0, p1 = PACE0[(ct, p)]
                                dma = nc.gpsimd.dma_start(
                                    xtiles[1][:, p0:p1, :, :],
                                    t_d[1, :, p0:p1, :, :],
                                )
                                _add_dep_helper(
                                    dma.ins, ev.ins, sync=True,
                                    reason="pace image-1 prefetch",
                                )
                    continue
                oh = 0
                for gi, R in enumerate(ROW_CHUNKS):
                    # Per-row-group slab [cout, ct, p, row, j], 5-deep
                    # pool: WAR against its own store is ~4 row-groups
                    # (~40us) away. Each (ct, plane-half) flushes to its
                    # contiguous block of o_d as soon as its planes are
                    # evacuated.
                    off = NP * NJ * oh
                    slab = mpool.tile([128, 2, NP, R, NJ], FP16, tag="m",
                                      name=f"m{b}_{gi}")
                    last_cast = None
                    for ct in range(2):
                        ps = []
                        for p in range(NP):
                            pst = psum.tile([128, R, NJ], F32, tag="ps")
                            ps.append(pst)
                            for kh in range(KH):
                                nc.tensor.matmul(
                                    pst[:],
                                    wr[:, ct, p, kh, :],
                                    xr[:, p, oh + kh : oh + kh + R, :],
                                    start=(kh == 0),
                                    stop=(kh == KH - 1),
                                )
                        # Evacuate: ACT p0..p4, DVE p5..p9 (fp32 -> fp16).
                        for p in range(5):
                            nc.scalar.copy(slab[:, ct, p], ps[p][:])
                        co0 = ct * 128
                        half = 5 * R * NJ
                        nc.sync.dma_start(
                            o_d[b, co0 : co0 + 128, off : off + half],
                            slab[:, ct, 0:5],
                        )
                        for p in range(5, NP):
                            last_cast = nc.vector.tensor_copy(
                                slab[:, ct, p], ps[p][:]
                            )
                        nc.sync.dma_start(
                            o_d[b, co0 : co0 + 128, off + half : off + 2 * half],
                            slab[:, ct, 5:NP],
                        )
                    oh += R
                    if not last_img:
                        for p0, p1 in PF_PLANES[gi]:
                            dma = nc.gpsimd.dma_start(
                                xtiles[b + 1][:, p0:p1, :, :],
                                t_d[b + 1, :, p0:p1, :, :],
                            )
                            _add_dep_helper(
                                dma.ins,
                                last_cast.ins,
                                sync=True,
                                reason="pace input prefetch vs compute",
                            )

    nc.compile()
    return nc


def _get_nc():
    if "nc" not in _CACHE:
        _CACHE["nc"] = _build_nc()
    return _CACHE["nc"]


LAST_RESULT = None


def _host_transform_x(x):
    """x[32,128,112,112] f32 -> t[32,128,10,112,14] fp16 (B^T x along W,
    plane-major)."""
    x = np.asarray(x, dtype=np.float32)
    bt = np.ascontiguousarray(BT_MAT.T, dtype=np.float32)  # [10 taps, 10 p]
    t = np.empty((B, CIN, NP, H, NJ), dtype=np.float16)
    for b0 in range(0, B, 8):
        xp = np.zeros((8, CIN, H, WPAD), dtype=np.float32)
        xp[:, :, :, :W] = x[b0 : b0 + 8]
        # D[b,c,h,j,k] = xp[b,c,h,8j+k]
        D = np.lib.stride_tricks.sliding_window_view(xp, NP, axis=3)[
            :, :, :, :: M_TILE
        ]  # [8, C, H, 14, 10]
        tc = D @ bt  # [8, C, H, 14, 10planes]
        t[b0 : b0 + 8] = tc.transpose(0, 1, 4, 2, 3)
    return t


def _host_combine(mblk):
    """Blocked m [B, COUT, NP*OH*NJ] fp16 -> out [B, COUT, OH, 110] f32.

    Un-blocks per row-group ([p, rows, j] contiguous per cout), then
    applies A^T along the plane axis."""
    at = np.ascontiguousarray(AT_MAT, dtype=np.float32)  # [8, 10]
    nb = mblk.shape[0]
    m = np.empty((nb, COUT, NP, OH, NJ), dtype=np.float16)
    oh = 0
    for R in ROW_CHUNKS:
        off = NP * NJ * oh
        blk = mblk[:, :, off : off + NP * R * NJ].reshape(nb, COUT, NP, R, NJ)
        m[:, :, :, oh : oh + R, :] = blk
        oh += R
    out = np.empty((nb, COUT, OH, OW), dtype=np.float32)
    for b in range(nb):
        mb = m[b].astype(np.float32)  # [COUT, 10, OH, 14]
        o = np.tensordot(mb, at, axes=([1], [1]))  # [COUT, OH, 14, 8]
        out[b] = o.reshape(COUT, OH, M_TILE * NJ)[:, :, :OW]
    return out


def kernel(x, weight, trace=False):
    global LAST_RESULT
    t = _host_transform_x(x)
    # weight [Cout,Cin,3,3] -> g'[cin, p, kh, cout] = sum_kw G[p,kw] w
    w64 = np.asarray(weight, dtype=np.float64)
    wt = np.einsum("pw,ochw->cpho", G_MAT, w64).astype(np.float16)
    # [cin, p, kh, cout] -> [ct, cin, p, kh, 128]
    wt = np.ascontiguousarray(
        wt.reshape(CIN, NP, KH, 2, 128).transpose(3, 0, 1, 2, 4)
    )

    nc = _get_nc()
    in_maps = [
        {"t": t[i * BPC : (i + 1) * BPC], "w": wt} for i in range(NCORES)
    ]
    res = run_bass_kernel_spmd(
        nc, in_maps, core_ids=list(range(NCORES)), trace=trace
    )
    LAST_RESULT = res
    m = np.concatenate([r["o"] for r in res.results], axis=0)
    return _host_combine(m)
